# revision 12
# baseline (speedup 1.0000x reference)
"""Trainium2 8-core kernel for nn_Attention_21345987461594 (v2).

Multi-head attention: B=2, S=4096, E=512, H=8 heads, D=64.
Sharding: 16 (batch, head) pairs -> 2 heads per core. No collectives: each
core computes a partial out-projection; the host sums 4 partials per batch.

v2 design (vs the v1 slot kernel):
- The attention core runs entirely in the PE's 64-row tiling mode: score
  matmuls (K=64) alternate row tiles T0 (head 0, SBUF rows 0-63) / T8
  (head 1, rows 64-127), and PV is split into two K=64 sub-matmuls (keys
  0-63 via T0 -> pvA, keys 64-127 via T8 -> pvB accumulators). Alternating
  row tiles execute concurrently on the PE (the 2nd MM of each pair is
  nearly free) and the uniform mode avoids 64<->128 tiling-mode drains.
- All K=128 work (qkv projection, V-layout transposes, out-projection) is
  grouped into small per-block bursts so each slot pays at most 2 mode
  switches.
- The softmax exp is split between the Scalar engine (true exp) and the
  Vector engine (Schraudolph: bf16 bits = trunc((x*log2e + 126.94)*128 + .5)
  computed as one f32 mult+add with int16 output, read back by the PE as
  bf16). GpSimd takes the SBUF-only tail multiplies and the denominator
  broadcast DMAs.
"""

import sys

if "/opt/trn_rl_repo" not in sys.path:
    sys.path.insert(0, "/opt/trn_rl_repo")

import numpy as np
import ml_dtypes

import concourse.bass as bass
import concourse.tile as tile
from concourse import bacc, mybir
from concourse.bass_utils import run_bass_kernel_spmd
from concourse.masks import make_identity

BF16 = mybir.dt.bfloat16
F32 = mybir.dt.float32
I16 = mybir.dt.int16

B, S, E, H = 2, 4096, 512, 8
D = E // H          # 64
HPC = 2             # heads per core
N_CORES = 8
QB = 512            # query block
N_QB = S // QB      # 8
CH = 128            # key chunk
N_CH = S // CH      # 32 chunks = 32 group-slots per block
VW = 80             # V' slot width: 64 dims + ones col + pad

LOG2E = 1.4426950408889634
# DVE Schraudolph exp: int16 bits = trunc(raw * S_DVE + C_DVE); bf16(bits)
# approximates exp(raw/8).  corr=-0.06 zeroes the mean error; +0.5 emulates
# round-to-nearest under the truncating f32->int16 store.
S_DVE = (1.0 / 8.0) * LOG2E * 128.0
C_DVE = (127.0 - 0.06) * 128.0 + 0.5

# exp engine per group-slot: 'A' = Scalar/ACT, 'D' = Vector/DVE (12/32 DVE)
PATTERN = ["A"] * N_CH
for _k in range(13):
    PATTERN[(_k * N_CH) // 13] = "D"

TAIL_OFFS = (0, 1, 2, 4, 6, 13, 15)


def _build():
    nc = bacc.Bacc("TRN2", target_bir_lowering=False)

    xt_ext = nc.declare_dram_parameter("xt", [E, S], BF16, isOutput=False)
    wqkv_ext = nc.declare_dram_parameter("wqkv", [E, 3 * HPC * D], BF16, isOutput=False)
    bqkv_ext = nc.declare_dram_parameter("bqkv", [3 * HPC * D, 1], F32, isOutput=False)
    wout_ext = nc.declare_dram_parameter("wout", [HPC * D, E], BF16, isOutput=False)
    out_ext = nc.declare_dram_parameter("out", [E, S], BF16, isOutput=True)
    dn_scr = [nc.dram_tensor(f"dnscr{i}", [HPC, QB], F32) for i in range(2)]

    FW = HPC * D  # 128

    with tile.TileContext(nc) as tc:
        with (
            tc.tile_pool(name="consts", bufs=1) as consts,
            tc.tile_pool(name="ptA_pool", bufs=10) as ptA_pool,
            tc.tile_pool(name="ptD_pool", bufs=8) as ptD_pool,
            tc.tile_pool(name="sm_pool", bufs=2) as sm_pool,
            tc.tile_pool(name="ot_pool", bufs=4) as ot_pool,
            tc.tile_pool(name="psum_sc", bufs=2, space="PSUM") as psum_sc,
            tc.tile_pool(name="psum_pv", bufs=1, space="PSUM") as psum_pv,
        ):
            # ---- persistent SBUF ----
            xt_sb = [consts.tile([128, S], BF16, name=f"xt{e}") for e in range(4)]
            wq_sb = [consts.tile([128, 3 * FW], BF16, name=f"wq{e}") for e in range(4)]
            wout_sb = consts.tile([128, E], BF16, name="wout")
            b_t = [consts.tile([128, 1], F32, name=f"bq{f}") for f in range(3)]
            qT = consts.tile([128, S], BF16, name="qT")
            kT = consts.tile([128, S], BF16, name="kT")
            vT = consts.tile([128, S], BF16, name="vT")
            Vp = consts.tile([128, N_CH * HPC * VW], BF16, name="Vp")
            ident_bf = consts.tile([128, 128], BF16, name="ident")
            ones_col = consts.tile([1, D], BF16, name="ones_col")

            # ---- loads / constants ----
            for e in range(4):
                nc.sync.dma_start(
                    out=xt_sb[e][:, 0:QB], in_=xt_ext[e * 128 : (e + 1) * 128, 0:QB]
                )
            for e in range(4):
                nc.sync.dma_start(out=wq_sb[e], in_=wqkv_ext[e * 128 : (e + 1) * 128, :])
            nc.sync.dma_start(out=wout_sb, in_=wout_ext[:, :])
            for f in range(3):
                nc.scalar.dma_start(out=b_t[f], in_=bqkv_ext[f * FW : (f + 1) * FW, :])
            make_identity(nc, ident_bf)
            nc.vector.memset(ones_col, 1.0)
            Vp_view = Vp.rearrange("p (s w) -> p s w", w=VW)
            nc.vector.memset(Vp_view[:, :, D : D + 1], 1.0)
            for tb in range(1, N_QB):
                for e in range(4):
                    nc.sync.dma_start(
                        out=xt_sb[e][:, tb * QB : (tb + 1) * QB],
                        in_=xt_ext[e * 128 : (e + 1) * 128, tb * QB : (tb + 1) * QB],
                    )

            dests = (qT, kT, vT)

            # ---- 128-mode burst pieces ----
            def proj(ft, tbs):
                ps = psum_sc.tile([128, 1024], F32, tag="sc", name=f"prj{ft}_{tbs[0]}")
                for e in range(4):
                    for i, tb in enumerate(tbs):
                        nc.tensor.matmul(
                            ps[:, i * QB : (i + 1) * QB],
                            lhsT=wq_sb[e][:, ft * FW : (ft + 1) * FW],
                            rhs=xt_sb[e][:, tb * QB : (tb + 1) * QB],
                            start=(e == 0),
                            stop=(e == 3),
                        )
                for i, tb in enumerate(tbs):
                    nc.vector.tensor_scalar_add(
                        out=dests[ft][:, tb * QB : (tb + 1) * QB],
                        in0=ps[:, i * QB : (i + 1) * QB],
                        scalar1=b_t[ft],
                    )

            def vbuild(c0):
                """Transpose 4 key-chunks of vT into V' slots."""
                tp = psum_sc.tile([128, 2048], BF16, tag="sc", name=f"tp{c0}")
                for i in range(4):
                    c = c0 + i
                    nc.tensor.transpose(
                        tp[:, i * 128 : (i + 1) * 128],
                        vT[:, c * 128 : (c + 1) * 128],
                        ident_bf,
                    )
                for i in range(4):
                    c = c0 + i
                    src = tp[:, i * 128 : (i + 1) * 128].rearrange(
                        "p (h d) -> p h d", h=2
                    )
                    nc.vector.tensor_copy(
                        out=Vp_view[:, 2 * c : 2 * c + 2, 0:D], in_=src
                    )

            # ---- per-slot emission ----
            def emit_scores(st, c):
                qb = st["qb"]
                sc = psum_sc.tile([128, 1024], F32, tag="sc", name=f"sc{qb}_{c}")
                for h in range(HPC):
                    nc.tensor.matmul(
                        sc[:, h * QB : (h + 1) * QB],
                        lhsT=kT[h * D : (h + 1) * D, c * CH : (c + 1) * CH],
                        rhs=qT[h * D : (h + 1) * D, qb * QB : (qb + 1) * QB],
                        start=True,
                        stop=True,
                    )
                st["sc"][c] = sc

            def emit_exp(st, c):
                qb = st["qb"]
                sc = st["sc"].pop(c)
                if PATTERN[c] == "A":
                    pt = ptA_pool.tile([128, 1024], BF16, tag="ptA", name=f"ptA{qb}_{c}")
                    nc.scalar.activation(
                        out=pt,
                        in_=sc,
                        func=mybir.ActivationFunctionType.Exp,
                        scale=0.125,
                    )
                    st["pts"][c] = pt
                else:
                    pt = ptD_pool.tile([128, 1024], I16, tag="ptD", name=f"ptD{qb}_{c}")
                    nc.vector.tensor_scalar(
                        out=pt,
                        in0=sc,
                        scalar1=S_DVE,
                        scalar2=C_DVE,
                        op0=mybir.AluOpType.mult,
                        op1=mybir.AluOpType.add,
                    )
                    st["pts"][c] = pt.bitcast(BF16)

            def emit_pv(st, c):
                if st["pv"] is None:
                    st["pv"] = [
                        psum_pv.tile(
                            [128, QB], F32, tag=f"pv{ab}{h}", name=f"pv{ab}{h}_{st['qb']}"
                        )
                        for ab in "AB"
                        for h in range(HPC)
                    ]
                pt = st["pts"].pop(c)
                pvA0, pvA1, pvB0, pvB1 = st["pv"]
                acc = ((pvA0, pvB0), (pvA1, pvB1))
                for h, sub in ((0, 0), (1, 1), (1, 0), (0, 1)):
                    slot_off = (2 * c + h) * VW
                    nc.tensor.matmul(
                        acc[h][sub][0 : D + 1, :],
                        lhsT=Vp[sub * 64 : sub * 64 + 64, slot_off : slot_off + D + 1],
                        rhs=pt[sub * 64 : sub * 64 + 64, h * QB : (h + 1) * QB],
                        start=(c == 0),
                        stop=(c == N_CH - 1),
                    )

            def tail_step(st, step):
                qb = st["qb"]
                if step == 0:
                    # merge A+B accumulators (B staged via SBUF: only one
                    # tensor-tensor operand may live in PSUM). Layout for the
                    # gpsimd tail:
                    #   pvm[0:65, 0:512]     = h0 dims + h0 denom (row 64)
                    #   pvm[64:128, 512:1024] = h1 dims
                    #   dn1[1,512]           = h1 denom
                    pvA0, pvA1, pvB0, pvB1 = st["pv"]
                    st["pvm"] = sm_pool.tile([128, 1024], F32, tag="pvm", name=f"pvm{qb}")
                    st["mgs"] = sm_pool.tile([128, 1024], F32, tag="mgs", name=f"mgs{qb}")
                    st["dn1"] = sm_pool.tile([1, QB], F32, tag="dn1", name=f"dn1_{qb}")
                    nc.scalar.copy(
                        out=st["mgs"][0 : D + 1, 0:QB], in_=pvB0[0 : D + 1, :]
                    )
                    nc.scalar.copy(
                        out=st["mgs"][0 : D + 1, QB : 2 * QB], in_=pvB1[0 : D + 1, :]
                    )
                    nc.vector.tensor_add(
                        out=st["pvm"][0 : D + 1, 0:QB],
                        in0=pvA0[0 : D + 1, :],
                        in1=st["mgs"][0 : D + 1, 0:QB],
                    )
                    nc.vector.tensor_add(
                        out=st["pvm"][D:128, QB : 2 * QB],
                        in0=pvA1[0:D, :],
                        in1=st["mgs"][0:D, QB : 2 * QB],
                    )
                    nc.vector.tensor_add(
                        out=st["dn1"],
                        in0=pvA1[D : D + 1, :],
                        in1=st["mgs"][D : D + 1, QB : 2 * QB],
                    )
                elif step == 1:
                    if qb == N_QB - 1:
                        st["dn_bf"] = [
                            sm_pool.tile(
                                [1, QB], BF16, tag=f"dnbf{h}", bufs=1,
                                name=f"dnbf{qb}_{h}",
                            )
                            for h in range(HPC)
                        ]
                        nc.vector.tensor_copy(
                            out=st["dn_bf"][0], in_=st["pvm"][D : D + 1, 0:QB]
                        )
                        nc.vector.tensor_copy(out=st["dn_bf"][1], in_=st["dn1"])
                        return
                    nc.sync.dma_start(
                        out=dn_scr[qb % 2][0:1, :], in_=st["pvm"][D : D + 1, 0:QB]
                    )
                    nc.sync.dma_start(out=dn_scr[qb % 2][1:2, :], in_=st["dn1"])
                elif step == 2:
                    if qb == N_QB - 1:
                        st["rcb"] = psum_pv.tile(
                            [128, QB], F32, tag="pvA0", name=f"rcb{qb}"
                        )
                        for h in range(HPC):
                            nc.tensor.matmul(
                                st["rcb"][h * D : (h + 1) * D, :],
                                lhsT=ones_col,
                                rhs=st["dn_bf"][h],
                                start=True,
                                stop=True,
                            )
                        return
                    st["dnb"] = sm_pool.tile([128, QB], F32, tag="dnb", name=f"dnb{qb}")
                    for h in range(HPC):
                        row = dn_scr[qb % 2][h : h + 1, :]
                        src = bass.AP(
                            tensor=row.tensor,
                            offset=row.offset,
                            ap=[[0, D]] + list(row.ap),
                        )
                        nc.gpsimd.dma_start(
                            out=st["dnb"][h * D : (h + 1) * D, :], in_=src
                        )
                elif step == 3:
                    st["rcp"] = sm_pool.tile([128, QB], F32, tag="rcp", name=f"rcp{qb}")
                    nc.vector.reciprocal_approx_fast(
                        out=st["rcp"],
                        in_=st["rcb"] if qb == N_QB - 1 else st["dnb"],
                    )
                elif step == 4:
                    st["attnT"] = sm_pool.tile(
                        [128, QB], BF16, tag="attnT", name=f"attnT{qb}"
                    )
                    eng = nc.vector if qb == N_QB - 1 else nc.gpsimd
                    eng.tensor_mul(
                        out=st["attnT"][0:D, :],
                        in0=st["pvm"][0:D, 0:QB],
                        in1=st["rcp"][0:D, :],
                    )
                    eng.tensor_mul(
                        out=st["attnT"][D:128, :],
                        in0=st["pvm"][D:128, QB : 2 * QB],
                        in1=st["rcp"][D:128, :],
                    )
                elif step == 5:
                    # out projection (128-mode; runs in a burst window)
                    st["op"] = [
                        psum_sc.tile([128, 1024], F32, tag="sc", name=f"op{qb}_{i}")
                        for i in range(2)
                    ]
                    for et in range(4):
                        nc.tensor.matmul(
                            st["op"][et // 2][:, (et % 2) * QB : (et % 2 + 1) * QB],
                            lhsT=wout_sb[:, et * 128 : (et + 1) * 128],
                            rhs=st["attnT"],
                            start=True,
                            stop=True,
                        )
                else:
                    for i in range(2):
                        ot = ot_pool.tile([128, 1024], BF16, tag="ot")
                        nc.vector.tensor_copy(out=ot, in_=st["op"][i])
                        for j in range(2):
                            et = 2 * i + j
                            nc.sync.dma_start(
                                out=out_ext[
                                    et * 128 : (et + 1) * 128, qb * QB : (qb + 1) * QB
                                ],
                                in_=ot[:, j * QB : (j + 1) * QB],
                            )

            # ---- schedule ----
            extras = [
                ("vbuild", 0),
                ("proj", 1, (2, 3)),
                ("vbuild", 4),
                ("proj", 2, (2, 3)),
                ("vbuild", 8),
                ("proj", 1, (4, 5)),
                ("vbuild", 12),
                ("proj", 2, (4, 5)),
                ("vbuild", 16),
                ("proj", 1, (6, 7)),
                ("vbuild", 20),
                ("proj", 2, (6, 7)),
                ("vbuild", 24),
                ("vbuild", 28),
                ("proj", 0, (1, 2)),
                ("proj", 0, (3, 4)),
                ("proj", 0, (5, 6)),
                ("proj", 0, (7,)),
            ]
            extras_v = []
            done = {"k": 1, "q": 0, "v": 1, "vb": 0}

            def do_extra(it):
                if it[0] == "proj":
                    _, ft, tbs = it
                    proj(ft, tbs)
                    key = {0: "q", 1: "k", 2: "v"}[ft]
                    done[key] = max(done[key], max(tbs))
                else:
                    vbuild(it[1])
                    done["vb"] = it[1] + 4

            def pop_extra():
                do_extra(extras.pop(0))

            def pop_extra_v():
                do_extra(extras_v.pop(0))

            slot = 0
            pvq = []
            tails = []

            def pump_pv():
                for _ in range(2):
                    if not pvq:
                        return
                    s2, c2, es = pvq[0]
                    if slot < es + 2 or done["vb"] < c2 + 1:
                        return
                    pvq.pop(0)
                    emit_pv(s2, c2)
                    if c2 == N_CH - 1:
                        offs = (
                            (0, 1, 2, 3, 4, 5, 6)
                            if s2["qb"] == N_QB - 1
                            else TAIL_OFFS
                        )
                        for k2, off in enumerate(offs):
                            tails.append((s2, k2, slot + off))

            def pump_tails(burst_ok):
                while tails and tails[0][2] <= slot:
                    s2, k2, _ = tails[0]
                    if k2 == 5 and not burst_ok:
                        return
                    tails.pop(0)
                    tail_step(s2, k2)

            # prologue (overlaps the input-DMA window)
            proj(1, (0, 1))
            proj(0, (0,))
            proj(2, (0, 1))

            all_slots = [(qb, c) for qb in range(N_QB) for c in range(N_CH)]
            sts = {}

            def ensure_scores(i):
                qb, c = all_slots[i]
                k_need = min(N_QB - 1, ((c + 1) * CH) // QB)
                while extras and (done["k"] < k_need or done["q"] < qb):
                    pop_extra()
                st = sts.get(qb)
                if st is None:
                    st = sts[qb] = {"qb": qb, "sc": {}, "pts": {}, "pv": None}
                emit_scores(st, c)

            ensure_scores(0)
            for i, (qb, c) in enumerate(all_slots):
                st = sts[qb]
                emit_exp(st, c)
                pvq.append((st, c, slot))
                if i + 1 < len(all_slots):
                    ensure_scores(i + 1)
                burst = False
                if c % 3 == 2 and extras:
                    pop_extra()
                    burst = True
                if c == 20 and qb < N_QB - 1:
                    while extras and done["q"] < qb + 1:
                        pop_extra()
                        burst = True
                pump_pv()
                pump_tails(burst or c % 3 == 2)
                slot += 1
            while extras:
                pop_extra()
            while extras_v:
                pop_extra_v()
            while pvq or tails:
                pump_pv()
                pump_tails(True)
                slot += 1

    nc.compile()
    return nc


_NC = None
LAST = {}


def _get_nc():
    global _NC
    if _NC is None:
        _NC = _build()
    return _NC


def prep_in_maps(x, w_qkv, b_qkv, w_out):
    bf = ml_dtypes.bfloat16
    in_maps = []
    for c in range(N_CORES):
        b = c // 4
        h0 = (c % 4) * HPC * D
        w_slice = np.concatenate(
            [w_qkv[:, j * E + h0 : j * E + h0 + HPC * D] for j in range(3)], axis=1
        )
        b_slice = np.concatenate(
            [b_qkv[j * E + h0 : j * E + h0 + HPC * D] for j in range(3)]
        )[:, None]
        in_maps.append(
            {
                "xt": np.ascontiguousarray(x[b].T).astype(bf),
                "wqkv": np.ascontiguousarray(w_slice).astype(bf),
                "bqkv": np.ascontiguousarray(b_slice.astype(np.float32)),
                "wout": np.ascontiguousarray(w_out[h0 : h0 + HPC * D, :]).astype(bf),
            }
        )
    return in_maps


def kernel(x, w_qkv, b_qkv, w_out, b_out):
    x = np.asarray(x, dtype=np.float32)
    w_qkv = np.asarray(w_qkv, dtype=np.float32)
    b_qkv = np.asarray(b_qkv, dtype=np.float32)
    w_out = np.asarray(w_out, dtype=np.float32)
    b_out = np.asarray(b_out, dtype=np.float32)

    in_maps = prep_in_maps(x, w_qkv, b_qkv, w_out)
    res = run_bass_kernel_spmd(_get_nc(), in_maps, list(range(N_CORES)))
    LAST["exec_time_ns"] = res.exec_time_ns
    LAST["res"] = res

    out = np.empty((B, S, E), dtype=np.float32)
    for b in range(B):
        acc = res.results[4 * b]["out"].astype(np.float32)
        for c in range(4 * b + 1, 4 * b + 4):
            acc = acc + res.results[c]["out"]
        out[b] = acc.T + b_out[None, :]
    return out
